# revision 15
# baseline (speedup 1.0000x reference)
"""Deformable conv net kernel for 8 TRN2 NeuronCores (data-parallel over batch).

Gather-x-first algorithm (per core, one batch sample):
  1. offsets = conv3x3(x, offset_w) + offset_b            (PE, bf16)
  2. per-pixel bilinear fields: quad base index (by,bx) + 4 slot weights
     folded with the clip-deviation masks                 (DVE, fp32)
  3. quad-gather x corners from a host-duplicated [HW, 2C] source:
     one 1KB item = all 4 bilinear corners of (pixel,tap) (SWDGE)
  4. tmp = gt_slot * w_slot (TensorScalarPtr), transpose-accumulated
     into S_k^T[c, pix] PSUM via identity matmuls         (DVE+PE)
  5. outT[o, pix] += W_k^T-style matmuls over c, 9 taps   (PE)
  6. outT += bias (per-partition) on ACT; host reshapes [8,128,64,64].

vs the previous matmul-first version this removes the 9.4MB Y round-trip
(write+no 1x1 convs), the 96 ACT copies and the 576 wide identity
accumulate matmuls; DMA drops from ~206us to ~135us busy per core.
"""
import os, sys

for _p in ("/opt/trn_rl_repo", "/root/.axon_site/_ro/trn_rl_repo"):
    if os.path.isdir(_p) and _p not in sys.path:
        sys.path.insert(0, _p)

import numpy as np
import ml_dtypes

import concourse.bass as bass
import concourse.mybir as mybir
from concourse import bacc, library_config
from concourse.tile import TileContext

BF16 = mybir.dt.bfloat16
F32 = mybir.dt.float32
I16 = mybir.dt.int16

B, C, H, W = 8, 128, 64, 64
O = 128
K = 3
K2 = 9
HW = H * W                 # 4096
NCH = HW // 128            # 32 pixel chunks of 128
NG = 4                     # pixel groups for the gather phase
CLG = NCH // NG            # 8 chunks per group
FDIM = NCH * K2            # 288, (c, k) col = c*9 + k
MAGIC = float(3 * 2 ** 22)

_MAX_WAITS = 1             # this walrus build rejects >1 sem wait per inst


def _split_excess_waits(nc):
    for f in nc.m.functions:
        for bb in f.blocks:
            new_insts = []
            for inst in bb.instructions:
                si = inst.sync_info
                if si is not None and si.on_wait and len(si.on_wait) > _MAX_WAITS:
                    waits = list(si.on_wait)
                    keep = waits[-_MAX_WAITS:]
                    spill = waits[:-_MAX_WAITS]
                    for j in range(0, len(spill), _MAX_WAITS):
                        chunk = spill[j:j + _MAX_WAITS]
                        nop = mybir.InstNoOp(
                            name=f"{inst.name}-wsp{j}",
                            engine=inst.engine,
                            ins=[], outs=[],
                            sync_info=mybir.SyncInfo(on_wait=chunk, on_update=[]),
                        )
                        nc.register_instruction(nop, overwrite=True)
                        new_insts.append(nop)
                    inst.sync_info = mybir.SyncInfo(
                        on_wait=keep, on_update=list(si.on_update or []))
                new_insts.append(inst)
            bb.instructions[:] = new_insts


def build_nc(gbufs=4, tbufs=8, act_every=0, debug=False, dbg_g=0, dbg_k=0):
    nc = bacc.Bacc()
    if debug:
        dbg_off = nc.dram_tensor("dbg_off", [18, HW], F32, kind="ExternalOutput")
        dbg_idx = nc.dram_tensor("dbg_idx", [128, FDIM], F32, kind="ExternalOutput")
        dbg_w = nc.dram_tensor("dbg_w", [128, 4 * FDIM], F32, kind="ExternalOutput")
        dbg_gt = nc.dram_tensor("dbg_gt", [128, CLG * 4 * C], F32, kind="ExternalOutput")
        dbg_s = nc.dram_tensor("dbg_s", [128, CLG * 128], F32, kind="ExternalOutput")
        dbg_sall = nc.dram_tensor("dbg_sall", [128, K2 * CLG * 128], BF16,
                                  kind="ExternalOutput")
        dbg_o = nc.dram_tensor("dbg_o", [128, CLG * 128], F32, kind="ExternalOutput")
        dbg_ixw = nc.dram_tensor("dbg_ixw", [128, 3 * 768], I16, kind="ExternalOutput")
    x_in = nc.dram_tensor("x_img", [C, HW], BF16, kind="ExternalInput")
    xq_in = nc.dram_tensor("xq", [HW, 2 * C], BF16, kind="ExternalInput")
    offw_in = nc.dram_tensor("offw", [C, K2 * 18], BF16, kind="ExternalInput")
    offb_in = nc.dram_tensor("offb", [18, 1], F32, kind="ExternalInput")
    wmain_in = nc.dram_tensor("wmain", [C, K2 * O], BF16, kind="ExternalInput")
    biasc_in = nc.dram_tensor("bias_c", [128, 1], F32, kind="ExternalInput")
    ybase_in = nc.dram_tensor("ybase", [128, FDIM], F32, kind="ExternalInput")
    xbase_in = nc.dram_tensor("xbase", [128, FDIM], F32, kind="ExternalInput")
    idf_in = nc.dram_tensor("identf", [128, 128], F32, kind="ExternalInput")
    idb_in = nc.dram_tensor("identb", [128, 128], BF16, kind="ExternalInput")
    out_dram = nc.dram_tensor("out", [O, HW], F32, kind="ExternalOutput")

    NKG = 3                     # idx scatter k-groups (3 taps each)
    VA = mybir.AluOpType

    with TileContext(nc) as tc:
        with tc.tile_pool(name="cst", bufs=1) as cst, \
             tc.tile_pool(name="fld", bufs=1) as fld, \
             tc.tile_pool(name="gth", bufs=gbufs) as gth, \
             tc.tile_pool(name="ssb", bufs=2) as ssb, \
             tc.tile_pool(name="osb", bufs=2) as osb, \
             tc.tile_pool(name="tmp", bufs=tbufs) as tmppool:

            nc.gpsimd.load_library(library_config.mlp)

            # Tiny SWDGE op up front: keeps bass's first-dynamic-DMA barrier
            # off the gather critical path.
            warm = cst.tile([16, 16], BF16, name="warm")
            nc.gpsimd.dma_start(warm[:, :], x_in[0:16, 0:16])

            # ---- constant / input loads ----
            offw_sb = cst.tile([C, K2 * 18], BF16, name="offw_sb")
            nc.sync.dma_start(offw_sb[:, :], offw_in[:, :])
            wmain_sb = cst.tile([C, K2 * O], BF16, name="wmain_sb")
            nc.sync.dma_start(wmain_sb[:, :], wmain_in[:, :])
            offb_sb = cst.tile([18, 1], F32, name="offb_sb")
            nc.sync.dma_start(offb_sb[:, :], offb_in[:, :])
            biasc_sb = cst.tile([128, 1], F32, name="biasc_sb")
            nc.sync.dma_start(biasc_sb[:, :], biasc_in[:, :])
            ybase_sb = cst.tile([128, FDIM], F32, name="ybase_sb")
            nc.sync.dma_start(ybase_sb[:, :], ybase_in[:, :])
            xbase_sb = cst.tile([128, FDIM], F32, name="xbase_sb")
            nc.sync.dma_start(xbase_sb[:, :], xbase_in[:, :])
            identf = cst.tile([128, 128], F32, name="identf")
            nc.sync.dma_start(identf[:, :], idf_in[:, :])
            identb = cst.tile([128, 128], BF16, name="identb")
            nc.sync.dma_start(identb[:, :], idb_in[:, :])

            # ---- row-padded image with 1-elem guards (contiguous conv rhs) ----
            XPR = (H + 4) * W
            xpr = cst.tile([C, XPR], BF16, name="xpr")
            nc.vector.memset(xpr[:, :], 0.0)
            nc.sync.dma_start(xpr[:, 1 + W: 1 + W + HW], x_in[:, :])

            psp_cm = tc.tile_pool(name="ps", bufs=2, space="PSUM")
            psp = psp_cm.__enter__()
            # ---- offset conv: offsets [18, HW] fp32 ----
            off_sb = fld.tile([18, HW], F32, name="off_sb")
            corr_ps = psp.tile([18, 2 * H], F32, name="corr_ps", tag="ph1ps")
            colL = xpr[:, 0:(H + 2) * W].rearrange("c (r w) -> c w r", w=W)
            colR = xpr[:, 1:1 + (H + 3) * W].rearrange("c (r w) -> c w r", w=W)
            for kh in range(3):
                nc.tensor.matmul(corr_ps[:, 0:H],
                                 offw_sb[:, (3 * kh) * 18:(3 * kh + 1) * 18],
                                 colL[:, 0, kh:kh + H],
                                 start=(kh == 0), stop=(kh == 2))
            for kh in range(3):
                nc.tensor.matmul(corr_ps[:, H:2 * H],
                                 offw_sb[:, (3 * kh + 2) * 18:(3 * kh + 3) * 18],
                                 colR[:, 0, kh + 1:kh + 1 + H],
                                 start=(kh == 0), stop=(kh == 2))
            for n in range(8):
                off_ps = psp.tile([18, 512], F32, name=f"offps{n}", tag="ph1ps")
                for k in range(K2):
                    kh, kw = k // 3, k % 3
                    base = 1 + (n * 8 + kh) * W + (kw - 1)
                    nc.tensor.matmul(off_ps[:, :], offw_sb[:, k * 18:(k + 1) * 18],
                                     xpr[:, base: base + 512],
                                     start=(k == 0), stop=(k == K2 - 1))
                nc.vector.tensor_scalar_add(off_sb[:, n * 512:(n + 1) * 512],
                                            off_ps[:, :], offb_sb[:, 0:1])
            offv = off_sb[:, :].rearrange("j (y x) -> j y x", x=W)
            nc.vector.tensor_tensor(
                offv[:, :, 0:1].rearrange("j y one -> j (y one)"),
                offv[:, :, 0:1].rearrange("j y one -> j (y one)"),
                corr_ps[:, 0:H], VA.subtract)
            nc.vector.tensor_tensor(
                offv[:, :, W - 1:W].rearrange("j y one -> j (y one)"),
                offv[:, :, W - 1:W].rearrange("j y one -> j (y one)"),
                corr_ps[:, H:2 * H], VA.subtract)

            # ---- transpose offsets to pixel-major: offT [128, 32*18] ----
            offT = fld.tile([128, NCH * 18], F32, name="offT")
            for cc in range(NCH):
                tr_ps = psp.tile([128, 18], F32, name=f"trps{cc}", tag="ph1ps")
                nc.tensor.transpose(tr_ps[:, :], off_sb[:, cc * 128:(cc + 1) * 128],
                                    identf[:18, :18])
                nc.scalar.copy(offT[:, cc * 18:(cc + 1) * 18], tr_ps[:, :])

            psp_cm.__exit__(None, None, None)

            # ---- bilinear quad fields (fp32, [128, (c,k)=288]) ----
            offT4 = offT[:, :].rearrange("p (c k two) -> p two c k", two=2, k=K2)
            yb3 = ybase_sb[:, :].rearrange("p (c k) -> p c k", k=K2)
            xb3 = xbase_sb[:, :].rearrange("p (c k) -> p c k", k=K2)

            def f3(name):
                t = fld.tile([128, FDIM], F32, name=name, tag=name)
                return t, t[:, :].rearrange("p (c k) -> p c k", k=K2)

            axr = {}
            for ax in ("y", "x"):
                s, s3 = f3(f"s_{ax}")
                base3 = yb3 if ax == "y" else xb3
                nc.vector.tensor_tensor(s3, offT4[:, 0 if ax == "y" else 1],
                                        base3, VA.add)
                r, _ = f3(f"r_{ax}")
                nc.vector.tensor_scalar_add(r[:, :], s[:, :], MAGIC)
                nc.vector.tensor_scalar_add(r[:, :], r[:, :], -MAGIC)
                gg, _ = f3(f"g_{ax}")
                nc.vector.tensor_tensor(gg[:, :], r[:, :], s[:, :], VA.is_gt)
                i0, _ = f3(f"i0_{ax}")
                nc.vector.tensor_tensor(i0[:, :], r[:, :], gg[:, :], VA.subtract)
                fr, _ = f3(f"fr_{ax}")
                nc.vector.tensor_tensor(fr[:, :], s[:, :], i0[:, :], VA.subtract)
                # validity of corners 0 (i0 in [0,63]) and 1 (i0 in [-1,62])
                v0, _ = f3(f"v0_{ax}")
                t2, _ = f3(f"t2_{ax}")
                nc.vector.tensor_scalar(v0[:, :], i0[:, :], 0.0, None, VA.is_ge)
                nc.vector.tensor_scalar(t2[:, :], i0[:, :], float(H - 1), None, VA.is_le)
                nc.vector.tensor_tensor(v0[:, :], v0[:, :], t2[:, :], VA.mult)
                v1, _ = f3(f"v1_{ax}")
                nc.vector.tensor_scalar(v1[:, :], i0[:, :], -1.0, None, VA.is_ge)
                nc.vector.tensor_scalar(t2[:, :], i0[:, :], float(H - 2), None, VA.is_le)
                nc.vector.tensor_tensor(v1[:, :], v1[:, :], t2[:, :], VA.mult)
                # w0 = (1-fr)*v0 ; w1 = fr*v1
                w0, _ = f3(f"w0_{ax}")
                nc.vector.tensor_scalar(w0[:, :], fr[:, :], -1.0, 1.0, VA.mult, VA.add)
                nc.vector.tensor_tensor(w0[:, :], w0[:, :], v0[:, :], VA.mult)
                w1, _ = f3(f"w1_{ax}")
                nc.vector.tensor_tensor(w1[:, :], fr[:, :], v1[:, :], VA.mult)
                # quad base and clip deviation: y keeps row 63 (B-slot content
                # is host-duplicated), x must stay <= 62 so the pair fits a row
                bmax = float(H - 1) if ax == "y" else float(W - 2)
                bb, _ = f3(f"b_{ax}")
                nc.vector.tensor_scalar(bb[:, :], i0[:, :], 0.0, bmax, VA.max, VA.min)
                dif, _ = f3(f"dif_{ax}")
                nc.vector.tensor_tensor(dif[:, :], bb[:, :], i0[:, :], VA.subtract)
                eq0, _ = f3(f"eq0_{ax}")
                nc.vector.tensor_scalar(eq0[:, :], dif[:, :], 0.0, None, VA.is_equal)
                eq1, _ = f3(f"eq1_{ax}")
                nc.vector.tensor_scalar(eq1[:, :], dif[:, :], 1.0, None, VA.is_equal)
                axr[ax] = dict(w0=w0, w1=w1, b=bb, dif=dif, eq0=eq0, eq1=eq1)

            y = axr["y"]; x = axr["x"]
            t1, _ = f3("t1")
            # y slot weights: T = w0*eq0 + w1*eq1 ; B = w1*eq0
            WyT, _ = f3("WyT")
            nc.vector.tensor_tensor(WyT[:, :], y["w0"][:, :], y["eq0"][:, :], VA.mult)
            nc.vector.tensor_tensor(t1[:, :], y["w1"][:, :], y["eq1"][:, :], VA.mult)
            nc.vector.tensor_tensor(WyT[:, :], WyT[:, :], t1[:, :], VA.add)
            WyB, _ = f3("WyB")
            nc.vector.tensor_tensor(WyB[:, :], y["w1"][:, :], y["eq0"][:, :], VA.mult)
            # x slot weights: L = w0*eq0 + w1*eq1 ; R = w1*eq0 + w0*eqm1
            eqm1, _ = f3("eqm1")
            nc.vector.tensor_scalar(eqm1[:, :], x["dif"][:, :], -1.0, None, VA.is_equal)
            WxL, _ = f3("WxL")
            nc.vector.tensor_tensor(WxL[:, :], x["w0"][:, :], x["eq0"][:, :], VA.mult)
            nc.vector.tensor_tensor(t1[:, :], x["w1"][:, :], x["eq1"][:, :], VA.mult)
            nc.vector.tensor_tensor(WxL[:, :], WxL[:, :], t1[:, :], VA.add)
            WxR, _ = f3("WxR")
            nc.vector.tensor_tensor(WxR[:, :], x["w1"][:, :], x["eq0"][:, :], VA.mult)
            nc.vector.tensor_tensor(t1[:, :], x["w0"][:, :], eqm1[:, :], VA.mult)
            nc.vector.tensor_tensor(WxR[:, :], WxR[:, :], t1[:, :], VA.add)
            # slot order matches quad item content: TL, BL, TR, BR
            wslot = []
            for Wx in (WxL, WxR):
                for Wy in (WyT, WyB):
                    wc, _ = f3(f"wc{len(wslot)}")
                    nc.vector.tensor_tensor(wc[:, :], Wy[:, :], Wx[:, :], VA.mult)
                    wslot.append(wc)
            # NOTE content order is (x-side outer, y inner): [TL, BL, TR, BR]
            wslot = [wslot[0], wslot[1], wslot[2], wslot[3]]

            # ---- quad row index fidx2 [128, (k,c)=288] = by*64 + bx ----
            by64, _ = f3("by64")
            nc.vector.tensor_scalar_mul(by64[:, :], y["b"][:, :], float(W))
            fidx2 = fld.tile([128, FDIM], F32, name="fidx2")
            fidx2_kc = fidx2[:, :].rearrange("p (k c) -> p c k", c=NCH)
            nc.vector.tensor_tensor(
                fidx2_kc,
                by64[:, :].rearrange("p (c k) -> p c k", k=K2),
                x["b"][:, :].rearrange("p (c k) -> p c k", k=K2), VA.add)
            fidxi2 = fld.tile([128, FDIM], I16, name="fidxi2")
            nc.vector.tensor_copy(fidxi2[:, :], fidx2[:, :])
            if debug:
                nc.sync.dma_start(dbg_off[:, :], off_sb[:, :])
                nc.sync.dma_start(dbg_idx[:, :], fidx2[:, :])
                for si in range(4):
                    nc.sync.dma_start(
                        dbg_w[:, si * FDIM:(si + 1) * FDIM], wslot[si][:, :])

            # ---- fold into SWDGE wrapped layout, pipelined by k-group ----
            # idxw col = ((k*NG+g)*CLG + cl)*8 + f ; item i = cl*128 + 16f + p'
            KPG = K2 // NKG
            idxw = []
            for kg in range(NKG):
                t = fld.tile([128, KPG * NG * CLG * 8], I16, name=f"idxw{kg}")
                idxw.append(t)
                dst_r = t[:, :].rearrange("p (kgcl f) -> p f kgcl", f=8)
                lo, hi = kg * KPG * NCH, (kg + 1) * KPG * NCH
                for f in range(8):
                    nc.sync.dma_start(dst_r[0:16, f],
                                      fidxi2[16 * f:16 * (f + 1), lo:hi])
                for f in range(1, 8):
                    nc.sync.dma_start(t[16 * f:16 * (f + 1), :], t[0:16, :])

            # ---- gather + weighted accumulate ----
            if debug:
                for kgd in range(NKG):
                    nc.sync.dma_start(dbg_ixw[:, kgd * 768:(kgd + 1) * 768],
                                      idxw[kgd][:, :])
            xq_src = xq_in[:, :]
            xq_pairs = bass.AP(tensor=xq_src.tensor, offset=xq_src.offset,
                               ap=[[2 * C, HW - 1], [1, 4 * C]])
            psS_cm = tc.tile_pool(name="psS", bufs=2, space="PSUM")
            psS = psS_cm.__enter__()
            psO_cm = tc.tile_pool(name="psO", bufs=2, space="PSUM")
            psO = psO_cm.__enter__()
            nact = 0
            for g in range(NG):
                s_sb = []
                for k in range(K2):
                    gt = gth.tile([128, CLG, 4 * C], BF16,
                                  name=f"g{g}_{k}", tag="gath")
                    base = (k * NG + g) * CLG * 8
                    kg = k // KPG
                    kbase = base - kg * KPG * NG * CLG * 8
                    nc.gpsimd.dma_gather(
                        gt[:, :, :], xq_pairs,
                        idxw[kg][:, kbase:kbase + CLG * 8],
                        CLG * 128, CLG * 128, 4 * C, elem_step=2 * C)
                    s_ps = psS.tile([128, CLG * 128], F32, name=f"sps{g}_{k}",
                                    tag="sps")
                    for cl in range(CLG):
                        c = g * CLG + cl
                        for slot in range(4):
                            tmp = tmppool.tile([128, 128], BF16,
                                               name=f"t{g}_{k}_{cl}_{slot}",
                                               tag="tmp")
                            nact += 1
                            if act_every and nact % act_every == 0:
                                nc.scalar.activation(
                                    tmp[:, :],
                                    gt[:, cl, slot * 128:(slot + 1) * 128],
                                    mybir.ActivationFunctionType.Copy,
                                    scale=wslot[slot][:, c * K2 + k:c * K2 + k + 1])
                            else:
                                nc.vector.tensor_scalar_mul(
                                    tmp[:, :],
                                    gt[:, cl, slot * 128:(slot + 1) * 128],
                                    wslot[slot][:, c * K2 + k:c * K2 + k + 1])
                            nc.tensor.matmul(
                                s_ps[:, cl * 128:(cl + 1) * 128],
                                tmp[:, :], identb[:, :],
                                start=(slot == 0), stop=(slot == 3))
                    sk = ssb.tile([128, CLG * 128], BF16, name=f"ssb{g}_{k}",
                                  tag=f"ssb{k}")
                    nc.scalar.copy(sk[:, :], s_ps[:, :])
                    s_sb.append(sk)
                    if debug and g == dbg_g:
                        nc.sync.dma_start(
                            dbg_sall[:, k * CLG * 128:(k + 1) * CLG * 128],
                            sk[:, :])
                    if debug and g == dbg_g and k == dbg_k:
                        dbg_gt_sb = fld.tile([128, CLG * 4 * C], F32, name="dbgt")
                        nc.vector.tensor_copy(
                            dbg_gt_sb[:, :],
                            gt[:, :, :].rearrange("p a b -> p (a b)"))
                        nc.sync.dma_start(dbg_gt[:, :], dbg_gt_sb[:, :])
                        dbg_s_sb = fld.tile([128, CLG * 128], F32, name="dbgs")
                        nc.vector.tensor_copy(dbg_s_sb[:, :], s_ps[:, :])
                        nc.sync.dma_start(dbg_s[:, :], dbg_s_sb[:, :])
                o_ps = psO.tile([128, CLG * 128], F32, name=f"ops{g}", tag="ops")
                for cl in range(CLG):
                    for k in range(K2):
                        nc.tensor.matmul(
                            o_ps[:, cl * 128:(cl + 1) * 128],
                            wmain_sb[:, k * O:(k + 1) * O],
                            s_sb[k][:, cl * 128:(cl + 1) * 128],
                            start=(k == 0), stop=(k == K2 - 1))
                ot = osb.tile([128, CLG * 128], F32, name=f"o{g}", tag="ot")
                if debug and g == dbg_g:
                    dbg_o_sb = fld.tile([128, CLG * 128], F32, name="dbgo")
                    nc.vector.tensor_copy(dbg_o_sb[:, :], o_ps[:, :])
                    nc.sync.dma_start(dbg_o[:, :], dbg_o_sb[:, :])
                nc.scalar.activation(ot[:, :], o_ps[:, :],
                                     mybir.ActivationFunctionType.Identity,
                                     bias=biasc_sb[:, 0:1])
                nc.sync.dma_start(out_dram[:, g * CLG * 128:(g + 1) * CLG * 128],
                                  ot[:, :])
            psO_cm.__exit__(None, None, None)
            psS_cm.__exit__(None, None, None)

    nc.compile()
    _split_excess_waits(nc)
    return nc


_NC_CACHE = None


def _get_nc():
    global _NC_CACHE
    if _NC_CACHE is None:
        _NC_CACHE = build_nc()
    return _NC_CACHE


def _host_inputs(x, offset_w, offset_b, weight, bias):
    bf = ml_dtypes.bfloat16
    offw = np.ascontiguousarray(
        offset_w.reshape(18, C, K2).transpose(1, 2, 0).reshape(C, K2 * 18)).astype(bf)
    wmain = np.ascontiguousarray(
        weight.reshape(O, C, K2).transpose(1, 2, 0).reshape(C, K2 * O)).astype(bf)
    offb = offset_b.reshape(18, 1).astype(np.float32)
    bias_c = bias.reshape(128, 1).astype(np.float32)
    pi = np.arange(128)
    cc = np.arange(NCH)
    kk = np.arange(K2)
    pix = cc[None, :, None] * 128 + pi[:, None, None]          # [128, 32, 1]
    ybase = (pix // W - 1 + (kk // 3)[None, None, :]).reshape(128, FDIM).astype(np.float32)
    xbase = (pix % W - 1 + (kk % 3)[None, None, :]).reshape(128, FDIM).astype(np.float32)
    identf = np.eye(128, dtype=np.float32)
    identb = np.eye(128, dtype=bf)
    shared = dict(offw=offw, wmain=wmain, offb=offb, bias_c=bias_c,
                  ybase=ybase, xbase=xbase, identf=identf, identb=identb)
    maps = []
    for b in range(B):
        xb = x[b].astype(bf)                                   # [C, H, W]
        m = dict(shared)
        m["x_img"] = np.ascontiguousarray(xb.reshape(C, HW))
        xt = np.ascontiguousarray(xb.transpose(1, 2, 0)).reshape(HW, C)
        xqv = np.zeros((HW, 2 * C), bf)
        xqv[:, 0:C] = xt
        xqv[:HW - W, C:2 * C] = xt[W:]
        m["xq"] = xqv
        maps.append(m)
    return maps


def kernel(x, offset_w, offset_b, weight, bias):
    from concourse.bass_utils import run_bass_kernel_spmd
    nc = _get_nc()
    in_maps = _host_inputs(np.asarray(x, np.float32), np.asarray(offset_w, np.float32),
                           np.asarray(offset_b, np.float32),
                           np.asarray(weight, np.float32), np.asarray(bias, np.float32))
    res = run_bass_kernel_spmd(nc, in_maps, core_ids=list(range(B)))
    out = np.stack([np.asarray(res.results[b]["out"], np.float32).reshape(O, H, W)
                    for b in range(B)])
    return out


# revision 19
# speedup vs baseline: 1.1023x; 1.1023x over previous
"""Deformable conv net kernel for 8 TRN2 NeuronCores (data-parallel over batch).

Gather-x-first algorithm (per core, one batch sample):
  1. offsets = conv3x3(x, offset_w) + offset_b            (PE, bf16)
  2. per-pixel bilinear fields: quad base index (by,bx) + 4 slot weights
     folded with the clip-deviation masks                 (DVE, fp32)
  3. quad-gather x corners from a host-duplicated [HW, 2C] source:
     one 1KB item = all 4 bilinear corners of (pixel,tap) (SWDGE)
  4. tmp = gt_slot * w_slot (TensorScalarPtr), transpose-accumulated
     into S_k^T[c, pix] PSUM via identity matmuls         (DVE+PE)
  5. outT[o, pix] += W_k^T-style matmuls over c, 9 taps   (PE)
  6. outT += bias (per-partition) on ACT; host reshapes [8,128,64,64].

vs the previous matmul-first version this removes the 9.4MB Y round-trip
(write+no 1x1 convs), the 96 ACT copies and the 576 wide identity
accumulate matmuls; DMA drops from ~206us to ~135us busy per core.
"""
import os, sys

for _p in ("/opt/trn_rl_repo", "/root/.axon_site/_ro/trn_rl_repo"):
    if os.path.isdir(_p) and _p not in sys.path:
        sys.path.insert(0, _p)

import numpy as np
import ml_dtypes

import concourse.bass as bass
import concourse.mybir as mybir
from concourse import bacc, library_config
from concourse.tile import TileContext

BF16 = mybir.dt.bfloat16
F32 = mybir.dt.float32
I16 = mybir.dt.int16

B, C, H, W = 8, 128, 64, 64
O = 128
K = 3
K2 = 9
HW = H * W                 # 4096
NCH = HW // 128            # 32 pixel chunks of 128
NG = 4                     # pixel groups for the gather phase
CLG = NCH // NG            # 8 chunks per group
FDIM = NCH * K2            # 288, (c, k) col = c*9 + k
MAGIC = float(3 * 2 ** 22)

_MAX_WAITS = 1             # this walrus build rejects >1 sem wait per inst


def _split_excess_waits(nc):
    for f in nc.m.functions:
        for bb in f.blocks:
            new_insts = []
            for inst in bb.instructions:
                si = inst.sync_info
                if si is not None and si.on_wait and len(si.on_wait) > _MAX_WAITS:
                    waits = list(si.on_wait)
                    keep = waits[-_MAX_WAITS:]
                    spill = waits[:-_MAX_WAITS]
                    for j in range(0, len(spill), _MAX_WAITS):
                        chunk = spill[j:j + _MAX_WAITS]
                        nop = mybir.InstNoOp(
                            name=f"{inst.name}-wsp{j}",
                            engine=inst.engine,
                            ins=[], outs=[],
                            sync_info=mybir.SyncInfo(on_wait=chunk, on_update=[]),
                        )
                        nc.register_instruction(nop, overwrite=True)
                        new_insts.append(nop)
                    inst.sync_info = mybir.SyncInfo(
                        on_wait=keep, on_update=list(si.on_update or []))
                new_insts.append(inst)
            bb.instructions[:] = new_insts


def build_nc(gbufs=4, tbufs=8, act_every=0, pool_every=0, debug=False, dbg_g=0, dbg_k=0):
    nc = bacc.Bacc()
    if debug:
        dbg_off = nc.dram_tensor("dbg_off", [18, HW], F32, kind="ExternalOutput")
        dbg_idx = nc.dram_tensor("dbg_idx", [128, FDIM], F32, kind="ExternalOutput")
        dbg_w = nc.dram_tensor("dbg_w", [128, 4 * FDIM], F32, kind="ExternalOutput")
        dbg_gt = nc.dram_tensor("dbg_gt", [128, CLG * 4 * C], F32, kind="ExternalOutput")
        dbg_s = nc.dram_tensor("dbg_s", [128, CLG * 128], F32, kind="ExternalOutput")
        dbg_sall = nc.dram_tensor("dbg_sall", [128, K2 * CLG * 128], BF16,
                                  kind="ExternalOutput")
        dbg_o = nc.dram_tensor("dbg_o", [128, CLG * 128], F32, kind="ExternalOutput")
        dbg_ixw = nc.dram_tensor("dbg_ixw", [128, 3 * 768], I16, kind="ExternalOutput")
    x_in = nc.dram_tensor("x_img", [C, HW], BF16, kind="ExternalInput")
    xq_in = nc.dram_tensor("xq", [HW, 2 * C], BF16, kind="ExternalInput")
    offw_in = nc.dram_tensor("offw", [C, K2 * 18], BF16, kind="ExternalInput")
    offb_in = nc.dram_tensor("offb", [18, 1], F32, kind="ExternalInput")
    wmain_in = nc.dram_tensor("wmain", [C, K2 * O], BF16, kind="ExternalInput")
    biasc_in = nc.dram_tensor("bias_c", [128, 1], F32, kind="ExternalInput")
    ybase_in = nc.dram_tensor("ybase", [128, FDIM], F32, kind="ExternalInput")
    xbase_in = nc.dram_tensor("xbase", [128, FDIM], F32, kind="ExternalInput")
    idf_in = nc.dram_tensor("identf", [128, 128], F32, kind="ExternalInput")
    idb_in = nc.dram_tensor("identb", [128, 128], BF16, kind="ExternalInput")
    out_dram = nc.dram_tensor("out", [O, HW], F32, kind="ExternalOutput")

    NKG = 3                     # idx scatter k-groups (3 taps each)
    VA = mybir.AluOpType

    with TileContext(nc) as tc:
        with tc.tile_pool(name="cst", bufs=1) as cst, \
             tc.tile_pool(name="fld", bufs=1) as fld, \
             tc.tile_pool(name="gth", bufs=gbufs) as gth, \
             tc.tile_pool(name="ssb", bufs=2) as ssb, \
             tc.tile_pool(name="osb", bufs=2) as osb, \
             tc.tile_pool(name="tmp", bufs=tbufs) as tmppool:

            nc.gpsimd.load_library(library_config.mlp)

            # Tiny SWDGE op up front: keeps bass's first-dynamic-DMA barrier
            # off the gather critical path.
            warm = cst.tile([16, 16], BF16, name="warm")
            nc.gpsimd.dma_start(warm[:, :], x_in[0:16, 0:16])

            # ---- constant / input loads ----
            offw_sb = cst.tile([C, K2 * 18], BF16, name="offw_sb")
            nc.sync.dma_start(offw_sb[:, :], offw_in[:, :])
            wmain_sb = cst.tile([C, K2 * O], BF16, name="wmain_sb")
            nc.sync.dma_start(wmain_sb[:, :], wmain_in[:, :])
            offb_sb = cst.tile([18, 1], F32, name="offb_sb")
            nc.sync.dma_start(offb_sb[:, :], offb_in[:, :])
            biasc_sb = cst.tile([128, 1], F32, name="biasc_sb")
            nc.sync.dma_start(biasc_sb[:, :], biasc_in[:, :])
            ybase_sb = cst.tile([128, FDIM], F32, name="ybase_sb")
            nc.sync.dma_start(ybase_sb[:, :], ybase_in[:, :])
            xbase_sb = cst.tile([128, FDIM], F32, name="xbase_sb")
            nc.sync.dma_start(xbase_sb[:, :], xbase_in[:, :])
            identf = cst.tile([128, 128], F32, name="identf")
            nc.sync.dma_start(identf[:, :], idf_in[:, :])
            identb = cst.tile([128, 128], BF16, name="identb")
            nc.sync.dma_start(identb[:, :], idb_in[:, :])

            # ---- row-padded image with 1-elem guards (contiguous conv rhs) ----
            XPR = (H + 4) * W
            xpr = cst.tile([C, XPR], BF16, name="xpr")
            nc.vector.memset(xpr[:, 0:1 + W], 0.0)
            nc.vector.memset(xpr[:, 1 + W + HW:XPR], 0.0)
            nc.sync.dma_start(xpr[:, 1 + W: 1 + W + HW], x_in[:, :])

            psp_cm = tc.tile_pool(name="ps", bufs=2, space="PSUM")
            psp = psp_cm.__enter__()
            # ---- offset conv: offsets [18, HW] fp32, pipelined per row-block ----
            off_sb = fld.tile([18, HW], F32, name="off_sb")
            offT = fld.tile([128, NCH * 18], F32, name="offT")
            corr_ps = psp.tile([18, 2 * H], F32, name="corr_ps", tag="corr")
            colL = xpr[:, 0:(H + 2) * W].rearrange("c (r w) -> c w r", w=W)
            colR = xpr[:, 1:1 + (H + 3) * W].rearrange("c (r w) -> c w r", w=W)
            for kh in range(3):
                nc.tensor.matmul(corr_ps[:, 0:H],
                                 offw_sb[:, (3 * kh) * 18:(3 * kh + 1) * 18],
                                 colL[:, 0, kh:kh + H],
                                 start=(kh == 0), stop=(kh == 2))
            for kh in range(3):
                nc.tensor.matmul(corr_ps[:, H:2 * H],
                                 offw_sb[:, (3 * kh + 2) * 18:(3 * kh + 3) * 18],
                                 colR[:, 0, kh + 1:kh + 1 + H],
                                 start=(kh == 0), stop=(kh == 2))
            offv = off_sb[:, :].rearrange("j (y x) -> j y x", x=W)
            for n in range(8):
                off_ps = psp.tile([18, 512], F32, name=f"offps{n}", tag="ph1ps")
                for k in range(K2):
                    kh, kw = k // 3, k % 3
                    base = 1 + (n * 8 + kh) * W + (kw - 1)
                    nc.tensor.matmul(off_ps[:, :], offw_sb[:, k * 18:(k + 1) * 18],
                                     xpr[:, base: base + 512],
                                     start=(k == 0), stop=(k == K2 - 1))
                nc.vector.tensor_scalar_add(off_sb[:, n * 512:(n + 1) * 512],
                                            off_ps[:, :], offb_sb[:, 0:1])
                nc.vector.tensor_tensor(
                    offv[:, 8 * n:8 * n + 8, 0:1].rearrange("j y one -> j (y one)"),
                    offv[:, 8 * n:8 * n + 8, 0:1].rearrange("j y one -> j (y one)"),
                    corr_ps[:, 8 * n:8 * n + 8], VA.subtract)
                nc.vector.tensor_tensor(
                    offv[:, 8 * n:8 * n + 8, W - 1:W].rearrange("j y one -> j (y one)"),
                    offv[:, 8 * n:8 * n + 8, W - 1:W].rearrange("j y one -> j (y one)"),
                    corr_ps[:, H + 8 * n:H + 8 * n + 8], VA.subtract)
                tr_ps = psp.tile([128, 72], F32, name=f"trps{n}", tag="trps")
                for j in range(4):
                    nc.tensor.transpose(tr_ps[:, 18 * j:18 * (j + 1)],
                                        off_sb[:, (4 * n + j) * 128:(4 * n + j + 1) * 128],
                                        identf[:18, :18])
                nc.scalar.copy(offT[:, 4 * n * 18:(4 * n + 4) * 18], tr_ps[:, :])

            psp_cm.__exit__(None, None, None)

            # ---- bilinear quad fields (fp32, [128, (c,k)=288]) ----
            offT4 = offT[:, :].rearrange("p (c k two) -> p two c k", two=2, k=K2)
            yb3 = ybase_sb[:, :].rearrange("p (c k) -> p c k", k=K2)
            xb3 = xbase_sb[:, :].rearrange("p (c k) -> p c k", k=K2)

            def f3(name):
                t = fld.tile([128, FDIM], F32, name=name, tag=name)
                return t, t[:, :].rearrange("p (c k) -> p c k", k=K2)

            axr = {}
            # ---- Phase A: index-only chain (both axes), then idx scatter ----
            for ax in ("y", "x"):
                s, s3 = f3(f"s_{ax}")
                base3 = yb3 if ax == "y" else xb3
                nc.vector.tensor_tensor(s3, offT4[:, 0 if ax == "y" else 1],
                                        base3, VA.add)
                r, _ = f3(f"r_{ax}")
                nc.vector.tensor_scalar_add(r[:, :], s[:, :], MAGIC)
                nc.vector.tensor_scalar_add(r[:, :], r[:, :], -MAGIC)
                gg, _ = f3(f"g_{ax}")
                nc.vector.tensor_tensor(gg[:, :], r[:, :], s[:, :], VA.is_gt)
                i0, _ = f3(f"i0_{ax}")
                nc.vector.tensor_tensor(i0[:, :], r[:, :], gg[:, :], VA.subtract)
                bmax = float(H - 1) if ax == "y" else float(W - 2)
                bb, _ = f3(f"b_{ax}")
                nc.vector.tensor_scalar(bb[:, :], i0[:, :], 0.0, bmax, VA.max, VA.min)
                axr[ax] = dict(s=s, i0=i0, b=bb)

            by64, _ = f3("by64")
            nc.vector.tensor_scalar_mul(by64[:, :], axr["y"]["b"][:, :], float(W))
            fidx2 = fld.tile([128, FDIM], F32, name="fidx2")
            fidx2_kc = fidx2[:, :].rearrange("p (k c) -> p c k", c=NCH)
            nc.vector.tensor_tensor(
                fidx2_kc,
                by64[:, :].rearrange("p (c k) -> p c k", k=K2),
                axr["x"]["b"][:, :].rearrange("p (c k) -> p c k", k=K2), VA.add)
            fidxi2 = fld.tile([128, FDIM], I16, name="fidxi2")
            nc.vector.tensor_copy(fidxi2[:, :], fidx2[:, :])

            # idx scatter early: gathers can start while weights compute
            KPG = K2 // NKG
            idxw = []
            for kg in range(NKG):
                t = fld.tile([128, KPG * NG * CLG * 8], I16, name=f"idxw{kg}")
                idxw.append(t)
                dst_r = t[:, :].rearrange("p (kgcl f) -> p f kgcl", f=8)
                lo, hi = kg * KPG * NCH, (kg + 1) * KPG * NCH
                for f in range(8):
                    nc.sync.dma_start(dst_r[0:16, f],
                                      fidxi2[16 * f:16 * (f + 1), lo:hi])
                for f in range(1, 8):
                    nc.sync.dma_start(t[16 * f:16 * (f + 1), :], t[0:16, :])

            # ---- Phase B: bilinear weight fields (overlap with gathers) ----
            for ax in ("y", "x"):
                s = axr[ax]["s"]; i0 = axr[ax]["i0"]; bb = axr[ax]["b"]
                fr, _ = f3(f"fr_{ax}")
                nc.vector.tensor_tensor(fr[:, :], s[:, :], i0[:, :], VA.subtract)
                v0, _ = f3(f"v0_{ax}")
                t2, _ = f3(f"t2_{ax}")
                nc.vector.tensor_scalar(v0[:, :], i0[:, :], 0.0, None, VA.is_ge)
                nc.vector.tensor_scalar(t2[:, :], i0[:, :], float(H - 1), None, VA.is_le)
                nc.vector.tensor_tensor(v0[:, :], v0[:, :], t2[:, :], VA.mult)
                v1, _ = f3(f"v1_{ax}")
                nc.vector.tensor_scalar(v1[:, :], i0[:, :], -1.0, None, VA.is_ge)
                nc.vector.tensor_scalar(t2[:, :], i0[:, :], float(H - 2), None, VA.is_le)
                nc.vector.tensor_tensor(v1[:, :], v1[:, :], t2[:, :], VA.mult)
                w0, _ = f3(f"w0_{ax}")
                nc.vector.tensor_scalar(w0[:, :], fr[:, :], -1.0, 1.0, VA.mult, VA.add)
                nc.vector.tensor_tensor(w0[:, :], w0[:, :], v0[:, :], VA.mult)
                w1, _ = f3(f"w1_{ax}")
                nc.vector.tensor_tensor(w1[:, :], fr[:, :], v1[:, :], VA.mult)
                dif, _ = f3(f"dif_{ax}")
                nc.vector.tensor_tensor(dif[:, :], bb[:, :], i0[:, :], VA.subtract)
                eq0, _ = f3(f"eq0_{ax}")
                nc.vector.tensor_scalar(eq0[:, :], dif[:, :], 0.0, None, VA.is_equal)
                eq1, _ = f3(f"eq1_{ax}")
                nc.vector.tensor_scalar(eq1[:, :], dif[:, :], 1.0, None, VA.is_equal)
                axr[ax].update(w0=w0, w1=w1, dif=dif, eq0=eq0, eq1=eq1)

            y = axr["y"]; x = axr["x"]
            t1, _ = f3("t1")
            WyT, _ = f3("WyT")
            nc.vector.tensor_tensor(WyT[:, :], y["w0"][:, :], y["eq0"][:, :], VA.mult)
            nc.vector.tensor_tensor(t1[:, :], y["w1"][:, :], y["eq1"][:, :], VA.mult)
            nc.vector.tensor_tensor(WyT[:, :], WyT[:, :], t1[:, :], VA.add)
            WyB, _ = f3("WyB")
            nc.vector.tensor_tensor(WyB[:, :], y["w1"][:, :], y["eq0"][:, :], VA.mult)
            eqm1, _ = f3("eqm1")
            nc.vector.tensor_scalar(eqm1[:, :], x["dif"][:, :], -1.0, None, VA.is_equal)
            WxL, _ = f3("WxL")
            nc.vector.tensor_tensor(WxL[:, :], x["w0"][:, :], x["eq0"][:, :], VA.mult)
            nc.vector.tensor_tensor(t1[:, :], x["w1"][:, :], x["eq1"][:, :], VA.mult)
            nc.vector.tensor_tensor(WxL[:, :], WxL[:, :], t1[:, :], VA.add)
            WxR, _ = f3("WxR")
            nc.vector.tensor_tensor(WxR[:, :], x["w1"][:, :], x["eq0"][:, :], VA.mult)
            nc.vector.tensor_tensor(t1[:, :], x["w0"][:, :], eqm1[:, :], VA.mult)
            nc.vector.tensor_tensor(WxR[:, :], WxR[:, :], t1[:, :], VA.add)
            wslot = []
            for Wx in (WxL, WxR):
                for Wy in (WyT, WyB):
                    wc, _ = f3(f"wc{len(wslot)}")
                    nc.vector.tensor_tensor(wc[:, :], Wy[:, :], Wx[:, :], VA.mult)
                    wslot.append(wc)
            if debug:
                nc.sync.dma_start(dbg_off[:, :], off_sb[:, :])
                nc.sync.dma_start(dbg_idx[:, :], fidx2[:, :])
                for si in range(4):
                    nc.sync.dma_start(
                        dbg_w[:, si * FDIM:(si + 1) * FDIM], wslot[si][:, :])
            if debug:
                for kgd in range(NKG):
                    nc.sync.dma_start(dbg_ixw[:, kgd * 768:(kgd + 1) * 768],
                                      idxw[kgd][:, :])
            # ---- gather + weighted accumulate ----
            if debug:
                for kgd in range(NKG):
                    nc.sync.dma_start(dbg_ixw[:, kgd * 768:(kgd + 1) * 768],
                                      idxw[kgd][:, :])
            xq_src = xq_in[:, :]
            xq_pairs = bass.AP(tensor=xq_src.tensor, offset=xq_src.offset,
                               ap=[[2 * C, HW - 1], [1, 4 * C]])
            psS_cm = tc.tile_pool(name="psS", bufs=2, space="PSUM")
            psS = psS_cm.__enter__()
            psO_cm = tc.tile_pool(name="psO", bufs=2, space="PSUM")
            psO = psO_cm.__enter__()
            nact = 0
            for g in range(NG):
                s_sb = []
                for k in range(K2):
                    gt = gth.tile([128, CLG, 4 * C], BF16,
                                  name=f"g{g}_{k}", tag="gath")
                    base = (k * NG + g) * CLG * 8
                    kg = k // KPG
                    kbase = base - kg * KPG * NG * CLG * 8
                    nc.gpsimd.dma_gather(
                        gt[:, :, :], xq_pairs,
                        idxw[kg][:, kbase:kbase + CLG * 8],
                        CLG * 128, CLG * 128, 4 * C, elem_step=2 * C)
                    s_ps = psS.tile([128, CLG * 128], F32, name=f"sps{g}_{k}",
                                    tag="sps")
                    for cl in range(CLG):
                        c = g * CLG + cl
                        for slot in range(4):
                            tmp = tmppool.tile([128, 128], BF16,
                                               name=f"t{g}_{k}_{cl}_{slot}",
                                               tag="tmp")
                            nact += 1
                            if act_every and nact % act_every == 0:
                                nc.scalar.activation(
                                    tmp[:, :],
                                    gt[:, cl, slot * 128:(slot + 1) * 128],
                                    mybir.ActivationFunctionType.Copy,
                                    scale=wslot[slot][:, c * K2 + k:c * K2 + k + 1])
                            elif pool_every and nact % pool_every == 1:
                                nc.gpsimd.tensor_scalar_mul(
                                    tmp[:, :],
                                    gt[:, cl, slot * 128:(slot + 1) * 128],
                                    wslot[slot][:, c * K2 + k:c * K2 + k + 1])
                            else:
                                nc.vector.tensor_scalar_mul(
                                    tmp[:, :],
                                    gt[:, cl, slot * 128:(slot + 1) * 128],
                                    wslot[slot][:, c * K2 + k:c * K2 + k + 1])
                            nc.tensor.matmul(
                                s_ps[:, cl * 128:(cl + 1) * 128],
                                tmp[:, :], identb[:, :],
                                start=(slot == 0), stop=(slot == 3))
                    sk = ssb.tile([128, CLG * 128], BF16, name=f"ssb{g}_{k}",
                                  tag=f"ssb{k}")
                    nc.scalar.copy(sk[:, :], s_ps[:, :])
                    s_sb.append(sk)
                    if debug and g == dbg_g:
                        nc.sync.dma_start(
                            dbg_sall[:, k * CLG * 128:(k + 1) * CLG * 128],
                            sk[:, :])
                    if debug and g == dbg_g and k == dbg_k:
                        dbg_gt_sb = fld.tile([128, CLG * 4 * C], F32, name="dbgt")
                        nc.vector.tensor_copy(
                            dbg_gt_sb[:, :],
                            gt[:, :, :].rearrange("p a b -> p (a b)"))
                        nc.sync.dma_start(dbg_gt[:, :], dbg_gt_sb[:, :])
                        dbg_s_sb = fld.tile([128, CLG * 128], F32, name="dbgs")
                        nc.vector.tensor_copy(dbg_s_sb[:, :], s_ps[:, :])
                        nc.sync.dma_start(dbg_s[:, :], dbg_s_sb[:, :])
                o_ps = psO.tile([128, CLG * 128], F32, name=f"ops{g}", tag="ops")
                for cl in range(CLG):
                    for k in range(K2):
                        nc.tensor.matmul(
                            o_ps[:, cl * 128:(cl + 1) * 128],
                            wmain_sb[:, k * O:(k + 1) * O],
                            s_sb[k][:, cl * 128:(cl + 1) * 128],
                            start=(k == 0), stop=(k == K2 - 1))
                ot = osb.tile([128, CLG * 128], F32, name=f"o{g}", tag="ot")
                if debug and g == dbg_g:
                    dbg_o_sb = fld.tile([128, CLG * 128], F32, name="dbgo")
                    nc.vector.tensor_copy(dbg_o_sb[:, :], o_ps[:, :])
                    nc.sync.dma_start(dbg_o[:, :], dbg_o_sb[:, :])
                nc.scalar.activation(ot[:, :], o_ps[:, :],
                                     mybir.ActivationFunctionType.Identity,
                                     bias=biasc_sb[:, 0:1])
                nc.sync.dma_start(out_dram[:, g * CLG * 128:(g + 1) * CLG * 128],
                                  ot[:, :])
            psO_cm.__exit__(None, None, None)
            psS_cm.__exit__(None, None, None)

    nc.compile()
    _split_excess_waits(nc)
    return nc


_NC_CACHE = None


def _get_nc():
    global _NC_CACHE
    if _NC_CACHE is None:
        _NC_CACHE = build_nc()
    return _NC_CACHE


def _host_inputs(x, offset_w, offset_b, weight, bias):
    bf = ml_dtypes.bfloat16
    offw = np.ascontiguousarray(
        offset_w.reshape(18, C, K2).transpose(1, 2, 0).reshape(C, K2 * 18)).astype(bf)
    wmain = np.ascontiguousarray(
        weight.reshape(O, C, K2).transpose(1, 2, 0).reshape(C, K2 * O)).astype(bf)
    offb = offset_b.reshape(18, 1).astype(np.float32)
    bias_c = bias.reshape(128, 1).astype(np.float32)
    pi = np.arange(128)
    cc = np.arange(NCH)
    kk = np.arange(K2)
    pix = cc[None, :, None] * 128 + pi[:, None, None]          # [128, 32, 1]
    ybase = (pix // W - 1 + (kk // 3)[None, None, :]).reshape(128, FDIM).astype(np.float32)
    xbase = (pix % W - 1 + (kk % 3)[None, None, :]).reshape(128, FDIM).astype(np.float32)
    identf = np.eye(128, dtype=np.float32)
    identb = np.eye(128, dtype=bf)
    shared = dict(offw=offw, wmain=wmain, offb=offb, bias_c=bias_c,
                  ybase=ybase, xbase=xbase, identf=identf, identb=identb)
    maps = []
    for b in range(B):
        xb = x[b].astype(bf)                                   # [C, H, W]
        m = dict(shared)
        m["x_img"] = np.ascontiguousarray(xb.reshape(C, HW))
        xt = np.ascontiguousarray(xb.transpose(1, 2, 0)).reshape(HW, C)
        xqv = np.zeros((HW, 2 * C), bf)
        xqv[:, 0:C] = xt
        xqv[:HW - W, C:2 * C] = xt[W:]
        m["xq"] = xqv
        maps.append(m)
    return maps


def kernel(x, offset_w, offset_b, weight, bias):
    from concourse.bass_utils import run_bass_kernel_spmd
    nc = _get_nc()
    in_maps = _host_inputs(np.asarray(x, np.float32), np.asarray(offset_w, np.float32),
                           np.asarray(offset_b, np.float32),
                           np.asarray(weight, np.float32), np.asarray(bias, np.float32))
    res = run_bass_kernel_spmd(nc, in_maps, core_ids=list(range(B)))
    out = np.stack([np.asarray(res.results[b]["out"], np.float32).reshape(O, H, W)
                    for b in range(B)])
    return out


# revision 20
# speedup vs baseline: 1.2972x; 1.1768x over previous
"""Deformable conv net kernel for 8 TRN2 NeuronCores (data-parallel over batch).

Gather-x-first algorithm (per core, one batch sample):
  1. offsets = conv3x3(x, offset_w) + offset_b            (PE, bf16)
  2. per-pixel bilinear fields: quad base index (by,bx) + 4 slot weights
     folded with the clip-deviation masks                 (DVE, fp32)
  3. quad-gather x corners from a host-duplicated [HW, 2C] source:
     one 1KB item = all 4 bilinear corners of (pixel,tap) (SWDGE)
  4. tmp = gt_slot * w_slot (TensorScalarPtr), transpose-accumulated
     into S_k^T[c, pix] PSUM via identity matmuls         (DVE+PE)
  5. outT[o, pix] += W_k^T-style matmuls over c, 9 taps   (PE)
  6. outT += bias (per-partition) on ACT; host reshapes [8,128,64,64].

vs the previous matmul-first version this removes the 9.4MB Y round-trip
(write+no 1x1 convs), the 96 ACT copies and the 576 wide identity
accumulate matmuls; DMA drops from ~206us to ~135us busy per core.
"""
import os, sys

for _p in ("/opt/trn_rl_repo", "/root/.axon_site/_ro/trn_rl_repo"):
    if os.path.isdir(_p) and _p not in sys.path:
        sys.path.insert(0, _p)

import numpy as np
import ml_dtypes

import concourse.bass as bass
import concourse.mybir as mybir
from concourse import bacc, library_config
from concourse.tile import TileContext

BF16 = mybir.dt.bfloat16
F32 = mybir.dt.float32
I16 = mybir.dt.int16

B, C, H, W = 8, 128, 64, 64
O = 128
K = 3
K2 = 9
HW = H * W                 # 4096
NCH = HW // 128            # 32 pixel chunks of 128
NG = 4                     # pixel groups for the gather phase
CLG = NCH // NG            # 8 chunks per group
FDIM = NCH * K2            # 288, (c, k) col = c*9 + k
MAGIC = float(3 * 2 ** 22)

_MAX_WAITS = 1             # this walrus build rejects >1 sem wait per inst


def _split_excess_waits(nc):
    for f in nc.m.functions:
        for bb in f.blocks:
            new_insts = []
            for inst in bb.instructions:
                si = inst.sync_info
                if si is not None and si.on_wait and len(si.on_wait) > _MAX_WAITS:
                    waits = list(si.on_wait)
                    keep = waits[-_MAX_WAITS:]
                    spill = waits[:-_MAX_WAITS]
                    for j in range(0, len(spill), _MAX_WAITS):
                        chunk = spill[j:j + _MAX_WAITS]
                        nop = mybir.InstNoOp(
                            name=f"{inst.name}-wsp{j}",
                            engine=inst.engine,
                            ins=[], outs=[],
                            sync_info=mybir.SyncInfo(on_wait=chunk, on_update=[]),
                        )
                        nc.register_instruction(nop, overwrite=True)
                        new_insts.append(nop)
                    inst.sync_info = mybir.SyncInfo(
                        on_wait=keep, on_update=list(si.on_update or []))
                new_insts.append(inst)
            bb.instructions[:] = new_insts


def build_nc(gbufs=6, tbufs=32, act_every=5, pool_every=5, debug=False, dbg_g=0, dbg_k=0):
    nc = bacc.Bacc()
    if debug:
        dbg_off = nc.dram_tensor("dbg_off", [18, HW], F32, kind="ExternalOutput")
        dbg_idx = nc.dram_tensor("dbg_idx", [128, FDIM], F32, kind="ExternalOutput")
        dbg_w = nc.dram_tensor("dbg_w", [128, 4 * FDIM], F32, kind="ExternalOutput")
        dbg_gt = nc.dram_tensor("dbg_gt", [128, CLG * 4 * C], F32, kind="ExternalOutput")
        dbg_s = nc.dram_tensor("dbg_s", [128, CLG * 128], F32, kind="ExternalOutput")
        dbg_sall = nc.dram_tensor("dbg_sall", [128, K2 * CLG * 128], BF16,
                                  kind="ExternalOutput")
        dbg_o = nc.dram_tensor("dbg_o", [128, CLG * 128], F32, kind="ExternalOutput")
        dbg_ixw = nc.dram_tensor("dbg_ixw", [128, 3 * 768], I16, kind="ExternalOutput")
    x_in = nc.dram_tensor("x_img", [C, HW], BF16, kind="ExternalInput")
    xq_in = nc.dram_tensor("xq", [HW, 2 * C], BF16, kind="ExternalInput")
    offw_in = nc.dram_tensor("offw", [C, K2 * 18], BF16, kind="ExternalInput")
    offb_in = nc.dram_tensor("offb", [18, 1], F32, kind="ExternalInput")
    wmain_in = nc.dram_tensor("wmain", [C, K2 * O], BF16, kind="ExternalInput")
    biasc_in = nc.dram_tensor("bias_c", [128, 1], F32, kind="ExternalInput")
    ybase_in = nc.dram_tensor("ybase", [128, FDIM], F32, kind="ExternalInput")
    xbase_in = nc.dram_tensor("xbase", [128, FDIM], F32, kind="ExternalInput")
    idf_in = nc.dram_tensor("identf", [128, 128], F32, kind="ExternalInput")
    idb_in = nc.dram_tensor("identb", [128, 128], BF16, kind="ExternalInput")
    out_dram = nc.dram_tensor("out", [O, HW], F32, kind="ExternalOutput")

    NKG = 3                     # idx scatter k-groups (3 taps each)
    VA = mybir.AluOpType

    with TileContext(nc) as tc:
        with tc.tile_pool(name="cst", bufs=1) as cst, \
             tc.tile_pool(name="fld", bufs=1) as fld, \
             tc.tile_pool(name="gth", bufs=gbufs) as gth, \
             tc.tile_pool(name="ssb", bufs=2) as ssb, \
             tc.tile_pool(name="osb", bufs=2) as osb, \
             tc.tile_pool(name="tmp", bufs=tbufs) as tmppool:

            nc.gpsimd.load_library(library_config.mlp)

            # Tiny SWDGE op up front: keeps bass's first-dynamic-DMA barrier
            # off the gather critical path.
            warm = cst.tile([16, 16], BF16, name="warm")
            nc.gpsimd.dma_start(warm[:, :], x_in[0:16, 0:16])

            # ---- constant / input loads ----
            offw_sb = cst.tile([C, K2 * 18], BF16, name="offw_sb")
            nc.sync.dma_start(offw_sb[:, :], offw_in[:, :])
            wmain_sb = cst.tile([C, K2 * O], BF16, name="wmain_sb")
            nc.sync.dma_start(wmain_sb[:, :], wmain_in[:, :])
            offb_sb = cst.tile([18, 1], F32, name="offb_sb")
            nc.sync.dma_start(offb_sb[:, :], offb_in[:, :])
            biasc_sb = cst.tile([128, 1], F32, name="biasc_sb")
            nc.sync.dma_start(biasc_sb[:, :], biasc_in[:, :])
            ybase_sb = cst.tile([128, FDIM], F32, name="ybase_sb")
            nc.sync.dma_start(ybase_sb[:, :], ybase_in[:, :])
            xbase_sb = cst.tile([128, FDIM], F32, name="xbase_sb")
            nc.sync.dma_start(xbase_sb[:, :], xbase_in[:, :])
            identf = cst.tile([128, 128], F32, name="identf")
            nc.sync.dma_start(identf[:, :], idf_in[:, :])
            identb = cst.tile([128, 128], BF16, name="identb")
            nc.sync.dma_start(identb[:, :], idb_in[:, :])

            # ---- row-padded image with 1-elem guards (contiguous conv rhs) ----
            XPR = (H + 4) * W
            xpr = cst.tile([C, XPR], BF16, name="xpr")
            nc.vector.memset(xpr[:, 0:1 + W], 0.0)
            nc.vector.memset(xpr[:, 1 + W + HW:XPR], 0.0)
            nc.sync.dma_start(xpr[:, 1 + W: 1 + W + HW], x_in[:, :])

            psp_cm = tc.tile_pool(name="ps", bufs=2, space="PSUM")
            psp = psp_cm.__enter__()
            # ---- offset conv: offsets [18, HW] fp32, pipelined per row-block ----
            off_sb = fld.tile([18, HW], F32, name="off_sb")
            offT = fld.tile([128, NCH * 18], F32, name="offT")
            corr_ps = psp.tile([18, 2 * H], F32, name="corr_ps", tag="corr")
            colL = xpr[:, 0:(H + 2) * W].rearrange("c (r w) -> c w r", w=W)
            colR = xpr[:, 1:1 + (H + 3) * W].rearrange("c (r w) -> c w r", w=W)
            for kh in range(3):
                nc.tensor.matmul(corr_ps[:, 0:H],
                                 offw_sb[:, (3 * kh) * 18:(3 * kh + 1) * 18],
                                 colL[:, 0, kh:kh + H],
                                 start=(kh == 0), stop=(kh == 2))
            for kh in range(3):
                nc.tensor.matmul(corr_ps[:, H:2 * H],
                                 offw_sb[:, (3 * kh + 2) * 18:(3 * kh + 3) * 18],
                                 colR[:, 0, kh + 1:kh + 1 + H],
                                 start=(kh == 0), stop=(kh == 2))
            offv = off_sb[:, :].rearrange("j (y x) -> j y x", x=W)
            for n in range(8):
                off_ps = psp.tile([18, 512], F32, name=f"offps{n}", tag="ph1ps")
                for k in range(K2):
                    kh, kw = k // 3, k % 3
                    base = 1 + (n * 8 + kh) * W + (kw - 1)
                    nc.tensor.matmul(off_ps[:, :], offw_sb[:, k * 18:(k + 1) * 18],
                                     xpr[:, base: base + 512],
                                     start=(k == 0), stop=(k == K2 - 1))
                nc.vector.tensor_scalar_add(off_sb[:, n * 512:(n + 1) * 512],
                                            off_ps[:, :], offb_sb[:, 0:1])
                nc.vector.tensor_tensor(
                    offv[:, 8 * n:8 * n + 8, 0:1].rearrange("j y one -> j (y one)"),
                    offv[:, 8 * n:8 * n + 8, 0:1].rearrange("j y one -> j (y one)"),
                    corr_ps[:, 8 * n:8 * n + 8], VA.subtract)
                nc.vector.tensor_tensor(
                    offv[:, 8 * n:8 * n + 8, W - 1:W].rearrange("j y one -> j (y one)"),
                    offv[:, 8 * n:8 * n + 8, W - 1:W].rearrange("j y one -> j (y one)"),
                    corr_ps[:, H + 8 * n:H + 8 * n + 8], VA.subtract)
                tr_ps = psp.tile([128, 72], F32, name=f"trps{n}", tag="trps")
                for j in range(4):
                    nc.tensor.transpose(tr_ps[:, 18 * j:18 * (j + 1)],
                                        off_sb[:, (4 * n + j) * 128:(4 * n + j + 1) * 128],
                                        identf[:18, :18])
                nc.scalar.copy(offT[:, 4 * n * 18:(4 * n + 4) * 18], tr_ps[:, :])

            psp_cm.__exit__(None, None, None)

            # ---- bilinear quad fields (fp32, [128, (c,k)=288]) ----
            offT4 = offT[:, :].rearrange("p (c k two) -> p two c k", two=2, k=K2)
            yb3 = ybase_sb[:, :].rearrange("p (c k) -> p c k", k=K2)
            xb3 = xbase_sb[:, :].rearrange("p (c k) -> p c k", k=K2)

            def f3(name):
                t = fld.tile([128, FDIM], F32, name=name, tag=name)
                return t, t[:, :].rearrange("p (c k) -> p c k", k=K2)

            axr = {}
            # ---- Phase A: index-only chain (both axes), then idx scatter ----
            for ax in ("y", "x"):
                s, s3 = f3(f"s_{ax}")
                base3 = yb3 if ax == "y" else xb3
                nc.vector.tensor_tensor(s3, offT4[:, 0 if ax == "y" else 1],
                                        base3, VA.add)
                r, _ = f3(f"r_{ax}")
                nc.vector.tensor_scalar_add(r[:, :], s[:, :], MAGIC)
                nc.vector.tensor_scalar_add(r[:, :], r[:, :], -MAGIC)
                gg, _ = f3(f"g_{ax}")
                nc.vector.tensor_tensor(gg[:, :], r[:, :], s[:, :], VA.is_gt)
                i0, _ = f3(f"i0_{ax}")
                nc.vector.tensor_tensor(i0[:, :], r[:, :], gg[:, :], VA.subtract)
                bmax = float(H - 1) if ax == "y" else float(W - 2)
                bb, _ = f3(f"b_{ax}")
                nc.vector.tensor_scalar(bb[:, :], i0[:, :], 0.0, bmax, VA.max, VA.min)
                axr[ax] = dict(s=s, i0=i0, b=bb)

            by64, _ = f3("by64")
            nc.vector.tensor_scalar_mul(by64[:, :], axr["y"]["b"][:, :], float(W))
            fidx2 = fld.tile([128, FDIM], F32, name="fidx2")
            fidx2_kc = fidx2[:, :].rearrange("p (k c) -> p c k", c=NCH)
            nc.vector.tensor_tensor(
                fidx2_kc,
                by64[:, :].rearrange("p (c k) -> p c k", k=K2),
                axr["x"]["b"][:, :].rearrange("p (c k) -> p c k", k=K2), VA.add)
            fidxi2 = fld.tile([128, FDIM], I16, name="fidxi2")
            nc.vector.tensor_copy(fidxi2[:, :], fidx2[:, :])

            # idx scatter early: gathers can start while weights compute
            KPG = K2 // NKG
            idxw = []
            for kg in range(NKG):
                t = fld.tile([128, KPG * NG * CLG * 8], I16, name=f"idxw{kg}")
                idxw.append(t)
                dst_r = t[:, :].rearrange("p (kgcl f) -> p f kgcl", f=8)
                lo, hi = kg * KPG * NCH, (kg + 1) * KPG * NCH
                for f in range(8):
                    nc.sync.dma_start(dst_r[0:16, f],
                                      fidxi2[16 * f:16 * (f + 1), lo:hi])
                for f in range(1, 8):
                    nc.sync.dma_start(t[16 * f:16 * (f + 1), :], t[0:16, :])

            # ---- Phase B: bilinear weight fields (overlap with gathers) ----
            for ax in ("y", "x"):
                s = axr[ax]["s"]; i0 = axr[ax]["i0"]; bb = axr[ax]["b"]
                fr, _ = f3(f"fr_{ax}")
                nc.vector.tensor_tensor(fr[:, :], s[:, :], i0[:, :], VA.subtract)
                v0, _ = f3(f"v0_{ax}")
                t2, _ = f3(f"t2_{ax}")
                nc.vector.tensor_scalar(v0[:, :], i0[:, :], 0.0, None, VA.is_ge)
                nc.vector.tensor_scalar(t2[:, :], i0[:, :], float(H - 1), None, VA.is_le)
                nc.vector.tensor_tensor(v0[:, :], v0[:, :], t2[:, :], VA.mult)
                v1, _ = f3(f"v1_{ax}")
                nc.vector.tensor_scalar(v1[:, :], i0[:, :], -1.0, None, VA.is_ge)
                nc.vector.tensor_scalar(t2[:, :], i0[:, :], float(H - 2), None, VA.is_le)
                nc.vector.tensor_tensor(v1[:, :], v1[:, :], t2[:, :], VA.mult)
                w0, _ = f3(f"w0_{ax}")
                nc.vector.tensor_scalar(w0[:, :], fr[:, :], -1.0, 1.0, VA.mult, VA.add)
                nc.vector.tensor_tensor(w0[:, :], w0[:, :], v0[:, :], VA.mult)
                w1, _ = f3(f"w1_{ax}")
                nc.vector.tensor_tensor(w1[:, :], fr[:, :], v1[:, :], VA.mult)
                dif, _ = f3(f"dif_{ax}")
                nc.vector.tensor_tensor(dif[:, :], bb[:, :], i0[:, :], VA.subtract)
                eq0, _ = f3(f"eq0_{ax}")
                nc.vector.tensor_scalar(eq0[:, :], dif[:, :], 0.0, None, VA.is_equal)
                eq1, _ = f3(f"eq1_{ax}")
                nc.vector.tensor_scalar(eq1[:, :], dif[:, :], 1.0, None, VA.is_equal)
                axr[ax].update(w0=w0, w1=w1, dif=dif, eq0=eq0, eq1=eq1)

            y = axr["y"]; x = axr["x"]
            t1, _ = f3("t1")
            WyT, _ = f3("WyT")
            nc.vector.tensor_tensor(WyT[:, :], y["w0"][:, :], y["eq0"][:, :], VA.mult)
            nc.vector.tensor_tensor(t1[:, :], y["w1"][:, :], y["eq1"][:, :], VA.mult)
            nc.vector.tensor_tensor(WyT[:, :], WyT[:, :], t1[:, :], VA.add)
            WyB, _ = f3("WyB")
            nc.vector.tensor_tensor(WyB[:, :], y["w1"][:, :], y["eq0"][:, :], VA.mult)
            eqm1, _ = f3("eqm1")
            nc.vector.tensor_scalar(eqm1[:, :], x["dif"][:, :], -1.0, None, VA.is_equal)
            WxL, _ = f3("WxL")
            nc.vector.tensor_tensor(WxL[:, :], x["w0"][:, :], x["eq0"][:, :], VA.mult)
            nc.vector.tensor_tensor(t1[:, :], x["w1"][:, :], x["eq1"][:, :], VA.mult)
            nc.vector.tensor_tensor(WxL[:, :], WxL[:, :], t1[:, :], VA.add)
            WxR, _ = f3("WxR")
            nc.vector.tensor_tensor(WxR[:, :], x["w1"][:, :], x["eq0"][:, :], VA.mult)
            nc.vector.tensor_tensor(t1[:, :], x["w0"][:, :], eqm1[:, :], VA.mult)
            nc.vector.tensor_tensor(WxR[:, :], WxR[:, :], t1[:, :], VA.add)
            wslot = []
            for Wx in (WxL, WxR):
                for Wy in (WyT, WyB):
                    wc, _ = f3(f"wc{len(wslot)}")
                    nc.vector.tensor_tensor(wc[:, :], Wy[:, :], Wx[:, :], VA.mult)
                    wslot.append(wc)
            if debug:
                nc.sync.dma_start(dbg_off[:, :], off_sb[:, :])
                nc.sync.dma_start(dbg_idx[:, :], fidx2[:, :])
                for si in range(4):
                    nc.sync.dma_start(
                        dbg_w[:, si * FDIM:(si + 1) * FDIM], wslot[si][:, :])
            if debug:
                for kgd in range(NKG):
                    nc.sync.dma_start(dbg_ixw[:, kgd * 768:(kgd + 1) * 768],
                                      idxw[kgd][:, :])
            # ---- gather + weighted accumulate ----
            if debug:
                for kgd in range(NKG):
                    nc.sync.dma_start(dbg_ixw[:, kgd * 768:(kgd + 1) * 768],
                                      idxw[kgd][:, :])
            xq_src = xq_in[:, :]
            xq_pairs = bass.AP(tensor=xq_src.tensor, offset=xq_src.offset,
                               ap=[[2 * C, HW - 1], [1, 4 * C]])
            psS_cm = tc.tile_pool(name="psS", bufs=2, space="PSUM")
            psS = psS_cm.__enter__()
            psO_cm = tc.tile_pool(name="psO", bufs=2, space="PSUM")
            psO = psO_cm.__enter__()
            nact = 0
            for g in range(NG):
                s_sb = []
                for k in range(K2):
                    gt = gth.tile([128, CLG, 4 * C], BF16,
                                  name=f"g{g}_{k}", tag="gath")
                    base = (k * NG + g) * CLG * 8
                    kg = k // KPG
                    kbase = base - kg * KPG * NG * CLG * 8
                    nc.gpsimd.dma_gather(
                        gt[:, :, :], xq_pairs,
                        idxw[kg][:, kbase:kbase + CLG * 8],
                        CLG * 128, CLG * 128, 4 * C, elem_step=2 * C)
                    s_ps = psS.tile([128, CLG * 128], F32, name=f"sps{g}_{k}",
                                    tag="sps")
                    for cl in range(CLG):
                        c = g * CLG + cl
                        for slot in range(4):
                            tmp = tmppool.tile([128, 128], BF16,
                                               name=f"t{g}_{k}_{cl}_{slot}",
                                               tag="tmp")
                            nact += 1
                            if act_every and nact % act_every == 0:
                                nc.scalar.activation(
                                    tmp[:, :],
                                    gt[:, cl, slot * 128:(slot + 1) * 128],
                                    mybir.ActivationFunctionType.Copy,
                                    scale=wslot[slot][:, c * K2 + k:c * K2 + k + 1])
                            elif pool_every and nact % pool_every == 1:
                                nc.gpsimd.tensor_scalar_mul(
                                    tmp[:, :],
                                    gt[:, cl, slot * 128:(slot + 1) * 128],
                                    wslot[slot][:, c * K2 + k:c * K2 + k + 1])
                            else:
                                nc.vector.tensor_scalar_mul(
                                    tmp[:, :],
                                    gt[:, cl, slot * 128:(slot + 1) * 128],
                                    wslot[slot][:, c * K2 + k:c * K2 + k + 1])
                            nc.tensor.matmul(
                                s_ps[:, cl * 128:(cl + 1) * 128],
                                tmp[:, :], identb[:, :],
                                start=(slot == 0), stop=(slot == 3))
                    sk = ssb.tile([128, CLG * 128], BF16, name=f"ssb{g}_{k}",
                                  tag=f"ssb{k}")
                    nc.scalar.copy(sk[:, :], s_ps[:, :])
                    s_sb.append(sk)
                    if debug and g == dbg_g:
                        nc.sync.dma_start(
                            dbg_sall[:, k * CLG * 128:(k + 1) * CLG * 128],
                            sk[:, :])
                    if debug and g == dbg_g and k == dbg_k:
                        dbg_gt_sb = fld.tile([128, CLG * 4 * C], F32, name="dbgt")
                        nc.vector.tensor_copy(
                            dbg_gt_sb[:, :],
                            gt[:, :, :].rearrange("p a b -> p (a b)"))
                        nc.sync.dma_start(dbg_gt[:, :], dbg_gt_sb[:, :])
                        dbg_s_sb = fld.tile([128, CLG * 128], F32, name="dbgs")
                        nc.vector.tensor_copy(dbg_s_sb[:, :], s_ps[:, :])
                        nc.sync.dma_start(dbg_s[:, :], dbg_s_sb[:, :])
                o_ps = psO.tile([128, CLG * 128], F32, name=f"ops{g}", tag="ops")
                for cl in range(CLG):
                    for k in range(K2):
                        nc.tensor.matmul(
                            o_ps[:, cl * 128:(cl + 1) * 128],
                            wmain_sb[:, k * O:(k + 1) * O],
                            s_sb[k][:, cl * 128:(cl + 1) * 128],
                            start=(k == 0), stop=(k == K2 - 1))
                ot = osb.tile([128, CLG * 128], F32, name=f"o{g}", tag="ot")
                if debug and g == dbg_g:
                    dbg_o_sb = fld.tile([128, CLG * 128], F32, name="dbgo")
                    nc.vector.tensor_copy(dbg_o_sb[:, :], o_ps[:, :])
                    nc.sync.dma_start(dbg_o[:, :], dbg_o_sb[:, :])
                nc.scalar.activation(ot[:, :], o_ps[:, :],
                                     mybir.ActivationFunctionType.Identity,
                                     bias=biasc_sb[:, 0:1])
                nc.sync.dma_start(out_dram[:, g * CLG * 128:(g + 1) * CLG * 128],
                                  ot[:, :])
            psO_cm.__exit__(None, None, None)
            psS_cm.__exit__(None, None, None)

    nc.compile()
    _split_excess_waits(nc)
    return nc


_NC_CACHE = None


def _get_nc():
    global _NC_CACHE
    if _NC_CACHE is None:
        _NC_CACHE = build_nc()
    return _NC_CACHE


def _host_inputs(x, offset_w, offset_b, weight, bias):
    bf = ml_dtypes.bfloat16
    offw = np.ascontiguousarray(
        offset_w.reshape(18, C, K2).transpose(1, 2, 0).reshape(C, K2 * 18)).astype(bf)
    wmain = np.ascontiguousarray(
        weight.reshape(O, C, K2).transpose(1, 2, 0).reshape(C, K2 * O)).astype(bf)
    offb = offset_b.reshape(18, 1).astype(np.float32)
    bias_c = bias.reshape(128, 1).astype(np.float32)
    pi = np.arange(128)
    cc = np.arange(NCH)
    kk = np.arange(K2)
    pix = cc[None, :, None] * 128 + pi[:, None, None]          # [128, 32, 1]
    ybase = (pix // W - 1 + (kk // 3)[None, None, :]).reshape(128, FDIM).astype(np.float32)
    xbase = (pix % W - 1 + (kk % 3)[None, None, :]).reshape(128, FDIM).astype(np.float32)
    identf = np.eye(128, dtype=np.float32)
    identb = np.eye(128, dtype=bf)
    shared = dict(offw=offw, wmain=wmain, offb=offb, bias_c=bias_c,
                  ybase=ybase, xbase=xbase, identf=identf, identb=identb)
    maps = []
    for b in range(B):
        xb = x[b].astype(bf)                                   # [C, H, W]
        m = dict(shared)
        m["x_img"] = np.ascontiguousarray(xb.reshape(C, HW))
        xt = np.ascontiguousarray(xb.transpose(1, 2, 0)).reshape(HW, C)
        xqv = np.zeros((HW, 2 * C), bf)
        xqv[:, 0:C] = xt
        xqv[:HW - W, C:2 * C] = xt[W:]
        m["xq"] = xqv
        maps.append(m)
    return maps


def kernel(x, offset_w, offset_b, weight, bias):
    from concourse.bass_utils import run_bass_kernel_spmd
    nc = _get_nc()
    in_maps = _host_inputs(np.asarray(x, np.float32), np.asarray(offset_w, np.float32),
                           np.asarray(offset_b, np.float32),
                           np.asarray(weight, np.float32), np.asarray(bias, np.float32))
    res = run_bass_kernel_spmd(nc, in_maps, core_ids=list(range(B)))
    out = np.stack([np.asarray(res.results[b]["out"], np.float32).reshape(O, H, W)
                    for b in range(B)])
    return out
